# revision 10
# baseline (speedup 1.0000x reference)
"""GPT-2 (V=32000, E=1024, H=1, HS=1024, L=4, T=1024, B=2) forward on 8 trn2 NeuronCores.

Sharding: 2 data-parallel groups of 4 cores (one per batch element). Within a
group, sequence is sharded "zigzag": rank r owns token blocks {r, 7-r} (128
tokens each).  Per layer each core computes q/k/v for its own 256 tokens,
all-gathers K^T and V inside its group, and computes attention rows + MLP for
its own tokens.  The LM head is vocab-sharded within the group (rank r owns
vocab columns [r*8000, (r+1)*8000)) after an all-gather of the final
activations.

All activations live in TRANSPOSED layout [feature-partition, token-free] so
every matmul is transpose-free; LayerNorm stats over the feature (partition)
dim are computed with ones-vector matmuls; per-token stats are applied via
partition_broadcast tiles.  Weights are cast to bf16 on host; accumulation in
fp32 PSUM.
"""
import sys

sys.path.insert(0, "/opt/trn_rl_repo")

import numpy as np
import ml_dtypes

import concourse.bass as bass
import concourse.tile as tile
from concourse import bacc, mybir
from concourse.bass_utils import run_bass_kernel_spmd

BF16 = mybir.dt.bfloat16
F32 = mybir.dt.float32
AF = mybir.ActivationFunctionType
ALU = mybir.AluOpType

V, E, H, HS, L, T, B = 32000, 1024, 1, 1024, 4, 1024, 2
EPS = 1e-5
SCALE = (H * HS) ** -0.5
NCORES = 8
GSZ = 4          # cores per group
NB = T // 128    # 8 token blocks per batch
VS = V // GSZ    # vocab shard per core = 8000
VCHUNK = 500     # vocab columns per psum tile (16 chunks)
NVC = VS // VCHUNK
ET = E // 128    # 8 feature tiles
MT = 4 * E // 128  # 32 mlp tiles
TOK = 256        # own tokens per core


def _ln_stats(nc, pool, psum, ones, src_bf, ntiles, sq_tile_pool, dim,
              eps_t=None):
    """Cross-partition LN stats over `ntiles` [128, TOK] bf16 tiles.

    Returns (a_vec, b_vec): [1, TOK] fp32 sbuf tiles with
    h = x * a - b  (before the per-feature affine)."""
    ps_sum = psum.tile([128, 512], F32, tag="ps")
    ps_sq = psum.tile([128, 512], F32, tag="ps")
    for t in range(ntiles):
        sq = sq_tile_pool.tile([128, TOK], BF16, tag="sq")
        nc.scalar.square(sq, src_bf(t))
        nc.tensor.matmul(ps_sum[0:1, 0:TOK], ones, src_bf(t),
                         start=(t == 0), stop=(t == ntiles - 1))
        nc.tensor.matmul(ps_sq[0:1, 0:TOK], ones, sq,
                         start=(t == 0), stop=(t == ntiles - 1))
    mean = pool.tile([1, TOK], F32, tag="stat")
    msq = pool.tile([1, TOK], F32, tag="stat")
    nc.scalar.mul(mean, ps_sum[0:1, 0:TOK], 1.0 / dim)
    nc.scalar.mul(msq, ps_sq[0:1, 0:TOK], 1.0 / dim)
    var = pool.tile([1, TOK], F32, tag="stat")
    nc.vector.tensor_mul(var, mean, mean)
    nc.vector.tensor_sub(var, msq, var)
    sd = pool.tile([1, TOK], F32, tag="stat")
    nc.scalar.activation(sd, var, AF.Sqrt, bias=eps_t)
    a_vec = pool.tile([1, TOK], F32, tag="stat")
    nc.vector.reciprocal(a_vec, sd)
    b_vec = pool.tile([1, TOK], F32, tag="stat")
    nc.vector.tensor_mul(b_vec, mean, a_vec)
    return a_vec, b_vec


def _bcast(nc, pool, vec, tag="bc"):
    out = pool.tile([128, TOK], F32, tag=tag)
    nc.gpsimd.partition_broadcast(out, vec)
    return out


def build_program():
    nc = bacc.Bacc("TRN2", target_bir_lowering=False, debug=False,
                   num_devices=NCORES)

    d_x0 = nc.dram_tensor("x0T", [E, TOK], F32, kind="ExternalInput").ap()
    d_mask = nc.dram_tensor("mask", [T, TOK], BF16, kind="ExternalInput").ap()
    d_wq = nc.dram_tensor("wq", [L, E, HS], BF16, kind="ExternalInput").ap()
    d_wk = nc.dram_tensor("wk", [L, E, HS], BF16, kind="ExternalInput").ap()
    d_wv = nc.dram_tensor("wv", [L, E, HS], BF16, kind="ExternalInput").ap()
    d_wo = nc.dram_tensor("wo", [L, HS, E], BF16, kind="ExternalInput").ap()
    d_w1 = nc.dram_tensor("w1", [L, E, 4 * E], BF16, kind="ExternalInput").ap()
    d_w2 = nc.dram_tensor("w2", [L, 4 * E, E], BF16, kind="ExternalInput").ap()
    # per-partition-tile reshaped affines/biases: [*, 128, ntiles]
    d_ln1 = nc.dram_tensor("ln1", [L, 2, 128, ET], F32, kind="ExternalInput").ap()
    d_ln2 = nc.dram_tensor("ln2", [L, 2, 128, ET], F32, kind="ExternalInput").ap()
    d_lnm = nc.dram_tensor("lnm", [L, 2, 128, MT], F32, kind="ExternalInput").ap()
    d_lnf = nc.dram_tensor("lnf", [2, 128, ET], F32, kind="ExternalInput").ap()
    d_bo = nc.dram_tensor("bo", [L, 128, ET], F32, kind="ExternalInput").ap()
    d_b1 = nc.dram_tensor("b1", [L, 128, MT], F32, kind="ExternalInput").ap()
    d_b2 = nc.dram_tensor("b2", [L, 128, ET], F32, kind="ExternalInput").ap()
    d_wlm = nc.dram_tensor("wlm", [E, VS], BF16, kind="ExternalInput").ap()
    d_blm = nc.dram_tensor("blm", [1, VS], F32, kind="ExternalInput").ap()
    d_out = nc.dram_tensor("logits", [T, VS], F32, kind="ExternalOutput").ap()

    with tile.TileContext(nc) as tc:
        import contextlib
        ctx = contextlib.ExitStack()
        with ctx:
            const = ctx.enter_context(tc.tile_pool(name="const", bufs=1))
            smalls = ctx.enter_context(tc.tile_pool(name="smalls", bufs=6))
            bcp = ctx.enter_context(tc.tile_pool(name="bcast", bufs=4))
            hp = ctx.enter_context(tc.tile_pool(name="acts", bufs=1))
            kqv = ctx.enter_context(tc.tile_pool(name="kqv", bufs=1))
            big = ctx.enter_context(tc.tile_pool(name="big", bufs=1))
            wts = ctx.enter_context(tc.tile_pool(name="wts", bufs=3))
            tmpp = ctx.enter_context(tc.tile_pool(name="tmpp", bufs=6))
            outp = ctx.enter_context(tc.tile_pool(name="outp", bufs=3))
            psum = ctx.enter_context(tc.tile_pool(name="psum", bufs=8,
                                                  space="PSUM"))
            dram = ctx.enter_context(tc.tile_pool(name="dram", bufs=1,
                                                  space="DRAM"))

            ones = const.tile([128, 1], BF16)
            nc.vector.memset(ones, 1.0)
            eps_t = const.tile([1, 1], F32)
            nc.vector.memset(eps_t, EPS)

            mask_sb = const.tile([128, NB, TOK], BF16)
            nc.sync.dma_start(out=mask_sb,
                              in_=d_mask.rearrange("(n p) q -> p n q", p=128))

            # residual stream x^T, fp32, [E, 256]
            xT = const.tile([128, ET, TOK], F32)
            nc.sync.dma_start(out=xT,
                              in_=d_x0.rearrange("(e p) q -> p e q", p=128))

            # gathered K^T [HS, T] and V [T, HS], bf16
            kt_g = big.tile([128, ET, T], BF16)
            v_g = big.tile([128, NB, HS], BF16)

            for l in range(L):
                ln1_sb = smalls.tile([128, 2, ET], F32, tag="ln1")
                nc.sync.dma_start(out=ln1_sb, in_=d_ln1[l].rearrange("a p t -> p a t"))
                ln2_sb = smalls.tile([128, 2, ET], F32, tag="ln2")
                nc.sync.dma_start(out=ln2_sb, in_=d_ln2[l].rearrange("a p t -> p a t"))
                lnm_sb = smalls.tile([128, 2, MT], F32, tag="lnm")
                nc.sync.dma_start(out=lnm_sb, in_=d_lnm[l].rearrange("a p t -> p a t"))
                bo_sb = smalls.tile([128, ET], F32, tag="bo")
                nc.sync.dma_start(out=bo_sb, in_=d_bo[l])
                b1_sb = smalls.tile([128, MT], F32, tag="b1")
                nc.sync.dma_start(out=b1_sb, in_=d_b1[l])
                b2_sb = smalls.tile([128, ET], F32, tag="b2")
                nc.sync.dma_start(out=b2_sb, in_=d_b2[l])

                # ---- LN1 -> h^T (bf16) ----
                xb = hp.tile([128, ET, TOK], BF16, tag="xb")
                for t in range(ET):
                    nc.scalar.copy(xb[:, t, :], xT[:, t, :])
                a_vec, b_vec = _ln_stats(nc, smalls, psum, ones,
                                         lambda t: xb[:, t, :], ET, tmpp, E, eps_t)
                ab = _bcast(nc, bcp, a_vec)
                bb = _bcast(nc, bcp, b_vec)
                hT = hp.tile([128, ET, TOK], BF16, tag="hT")
                for t in range(ET):
                    t1 = tmpp.tile([128, TOK], F32, tag="t1")
                    nc.vector.tensor_mul(t1, xT[:, t, :], ab)
                    t2 = tmpp.tile([128, TOK], F32, tag="t2")
                    nc.vector.tensor_sub(t2, t1, bb)
                    nc.vector.tensor_scalar(hT[:, t, :], t2,
                                            ln1_sb[:, 0, t:t + 1],
                                            ln1_sb[:, 1, t:t + 1],
                                            ALU.mult, ALU.add)

                # ---- k^T = (h Wk)^T, q^T likewise; v in row layout ----
                kT = kqv.tile([128, ET, TOK], BF16, tag="kT")
                qT = kqv.tile([128, ET, TOK], BF16, tag="qT")
                for half in range(2):
                    wk_sb = wts.tile([128, ET, HS // 2], BF16, tag="w")
                    nc.sync.dma_start(
                        out=wk_sb,
                        in_=d_wk[l][:, half * 512:(half + 1) * 512]
                        .rearrange("(e p) m -> p e m", p=128))
                    for m in range(4):
                        ps = psum.tile([128, 512], F32, tag="ps")
                        for e in range(ET):
                            nc.tensor.matmul(
                                ps[:, 0:TOK],
                                wk_sb[:, e, m * 128:(m + 1) * 128],
                                hT[:, e, :],
                                start=(e == 0), stop=(e == ET - 1))
                        nc.vector.tensor_copy(out=kT[:, half * 4 + m, :],
                                              in_=ps[:, 0:TOK])
                for half in range(2):
                    wq_sb = wts.tile([128, ET, HS // 2], BF16, tag="w")
                    nc.sync.dma_start(
                        out=wq_sb,
                        in_=d_wq[l][:, half * 512:(half + 1) * 512]
                        .rearrange("(e p) m -> p e m", p=128))
                    for m in range(4):
                        ps = psum.tile([128, 512], F32, tag="ps")
                        for e in range(ET):
                            nc.tensor.matmul(
                                ps[:, 0:TOK],
                                wq_sb[:, e, m * 128:(m + 1) * 128],
                                hT[:, e, :],
                                start=(e == 0), stop=(e == ET - 1))
                        nc.vector.tensor_copy(out=qT[:, half * 4 + m, :],
                                              in_=ps[:, 0:TOK])
                v_sb = kqv.tile([128, 2, HS], BF16, tag="v")
                for half in range(2):
                    wv_sb = wts.tile([128, ET, HS // 2], BF16, tag="w")
                    nc.sync.dma_start(
                        out=wv_sb,
                        in_=d_wv[l][:, half * 512:(half + 1) * 512]
                        .rearrange("(e p) m -> p e m", p=128))
                    for blk in range(2):
                        ps = psum.tile([128, 512], F32, tag="ps")
                        for e in range(ET):
                            nc.tensor.matmul(
                                ps,
                                hT[:, e, blk * 128:(blk + 1) * 128],
                                wv_sb[:, e, :],
                                start=(e == 0), stop=(e == ET - 1))
                        nc.vector.tensor_copy(
                            out=v_sb[:, blk, half * 512:(half + 1) * 512],
                            in_=ps)

                # ---- all-gather K^T and V within group of 4 ----
                kb_in = dram.tile([E, TOK], BF16)
                kb_out = dram.tile([GSZ * E, TOK], BF16)
                nc.gpsimd.dma_start(
                    out=kb_in.rearrange("(e p) q -> p e q", p=128), in_=kT)
                nc.gpsimd.collective_compute(
                    "AllGather", ALU.bypass,
                    replica_groups=[[0, 1, 2, 3], [4, 5, 6, 7]],
                    ins=[kb_in[:].opt()], outs=[kb_out[:].opt()])
                vb_in = dram.tile([TOK, HS], BF16)
                vb_out = dram.tile([GSZ * TOK, HS], BF16)
                nc.gpsimd.dma_start(
                    out=vb_in.rearrange("(b p) h -> p b h", p=128), in_=v_sb)
                nc.gpsimd.collective_compute(
                    "AllGather", ALU.bypass,
                    replica_groups=[[0, 1, 2, 3], [4, 5, 6, 7]],
                    ins=[vb_in[:].opt()], outs=[vb_out[:].opt()])
                # read back gathered K^T: rank r tile t -> cols r*256..
                for r in range(GSZ):
                    for t in range(ET):
                        nc.gpsimd.dma_start(
                            out=kt_g[:, t, r * TOK:(r + 1) * TOK],
                            in_=kb_out[r * E + t * 128: r * E + (t + 1) * 128, :])
                for kb in range(NB):
                    nc.gpsimd.dma_start(
                        out=v_g[:, kb, :],
                        in_=vb_out[kb * 128:(kb + 1) * 128, :])

                # ---- scores^T, masked exp, normalizer, o^T ----
                aT = hp.tile([128, NB, TOK], BF16, tag="aT")
                ps_r = psum.tile([128, 512], F32, tag="ps")
                for kb in range(NB):
                    ps = psum.tile([128, 512], F32, tag="ps")
                    for t in range(ET):
                        nc.tensor.matmul(
                            ps[:, 0:TOK],
                            kt_g[:, t, kb * 128:(kb + 1) * 128],
                            qT[:, t, :],
                            start=(t == 0), stop=(t == ET - 1))
                    nc.scalar.activation(aT[:, kb, :], ps[:, 0:TOK],
                                         AF.Exp, scale=SCALE)
                    nc.vector.tensor_mul(aT[:, kb, :], aT[:, kb, :],
                                         mask_sb[:, kb, :])
                    nc.tensor.matmul(ps_r[0:1, 0:TOK], ones, aT[:, kb, :],
                                     start=(kb == 0), stop=(kb == NB - 1))
                rinv = smalls.tile([1, TOK], F32, tag="stat")
                nc.vector.reciprocal(rinv, ps_r[0:1, 0:TOK])
                rb = _bcast(nc, bcp, rinv)
                oT = kqv.tile([128, ET, TOK], BF16, tag="oT")
                for m in range(ET):
                    ps = psum.tile([128, 512], F32, tag="ps")
                    for kb in range(NB):
                        nc.tensor.matmul(
                            ps[:, 0:TOK],
                            v_g[:, kb, m * 128:(m + 1) * 128],
                            aT[:, kb, :],
                            start=(kb == 0), stop=(kb == NB - 1))
                    nc.vector.tensor_mul(oT[:, m, :], ps[:, 0:TOK], rb)

                # ---- x += o Wo + bo ----
                for half in range(2):
                    wo_sb = wts.tile([128, ET, E // 2], BF16, tag="w")
                    nc.sync.dma_start(
                        out=wo_sb,
                        in_=d_wo[l][:, half * 512:(half + 1) * 512]
                        .rearrange("(h p) m -> p h m", p=128))
                    for m in range(4):
                        et = half * 4 + m
                        ps = psum.tile([128, 512], F32, tag="ps")
                        for h in range(ET):
                            nc.tensor.matmul(
                                ps[:, 0:TOK],
                                wo_sb[:, h, m * 128:(m + 1) * 128],
                                oT[:, h, :],
                                start=(h == 0), stop=(h == ET - 1))
                        t1 = tmpp.tile([128, TOK], F32, tag="t1")
                        nc.vector.tensor_scalar(t1, ps[:, 0:TOK],
                                                bo_sb[:, et:et + 1], None,
                                                ALU.add)
                        nc.vector.tensor_add(xT[:, et, :], xT[:, et, :], t1)

                # ---- LN2 -> h2^T ----
                xb2 = hp.tile([128, ET, TOK], BF16, tag="xb")
                for t in range(ET):
                    nc.scalar.copy(xb2[:, t, :], xT[:, t, :])
                a_vec, b_vec = _ln_stats(nc, smalls, psum, ones,
                                         lambda t: xb2[:, t, :], ET, tmpp, E, eps_t)
                ab = _bcast(nc, bcp, a_vec)
                bb = _bcast(nc, bcp, b_vec)
                h2T = hp.tile([128, ET, TOK], BF16, tag="hT")
                for t in range(ET):
                    t1 = tmpp.tile([128, TOK], F32, tag="t1")
                    nc.vector.tensor_mul(t1, xT[:, t, :], ab)
                    t2 = tmpp.tile([128, TOK], F32, tag="t2")
                    nc.vector.tensor_sub(t2, t1, bb)
                    nc.vector.tensor_scalar(h2T[:, t, :], t2,
                                            ln2_sb[:, 0, t:t + 1],
                                            ln2_sb[:, 1, t:t + 1],
                                            ALU.mult, ALU.add)

                # ---- u^T = (h2 W1 + b1)^T ----
                uT = big.tile([128, MT, TOK], BF16, tag="uT")
                for ch in range(8):   # 8 chunks of 512 mlp cols (4 m-tiles)
                    w1_sb = wts.tile([128, ET, 512], BF16, tag="w")
                    nc.sync.dma_start(
                        out=w1_sb,
                        in_=d_w1[l][:, ch * 512:(ch + 1) * 512]
                        .rearrange("(e p) m -> p e m", p=128))
                    for m in range(4):
                        mt = ch * 4 + m
                        ps = psum.tile([128, 512], F32, tag="ps")
                        for e in range(ET):
                            nc.tensor.matmul(
                                ps[:, 0:TOK],
                                w1_sb[:, e, m * 128:(m + 1) * 128],
                                hT if False else h2T[:, e, :],
                                start=(e == 0), stop=(e == ET - 1))
                        nc.vector.tensor_scalar(uT[:, mt, :], ps[:, 0:TOK],
                                                b1_sb[:, mt:mt + 1], None,
                                                ALU.add)

                # ---- m^T = gelu(lnm(u)) ----
                a_vec, b_vec = _ln_stats(nc, smalls, psum, ones,
                                         lambda t: uT[:, t, :], MT, tmpp, 4 * E, eps_t)
                ab = _bcast(nc, bcp, a_vec)
                bb = _bcast(nc, bcp, b_vec)
                mT = big.tile([128, MT, TOK], BF16, tag="mT")
                for t in range(MT):
                    t1 = tmpp.tile([128, TOK], F32, tag="t1")
                    nc.vector.tensor_mul(t1, uT[:, t, :], ab)
                    t2 = tmpp.tile([128, TOK], F32, tag="t2")
                    nc.vector.tensor_sub(t2, t1, bb)
                    nc.scalar.activation(mT[:, t, :], t2, AF.Gelu,
                                         bias=lnm_sb[:, 1, t:t + 1],
                                         scale=lnm_sb[:, 0, t:t + 1])

                # ---- x += m W2 + b2 (m-major accumulation) ----
                ps_w2 = [psum.tile([128, 512], F32, tag="ps", name=f"psw2_{e}")
                         for e in range(ET)]
                for ch in range(8):   # 8 chunks of 512 contraction rows
                    w2_sb = wts.tile([128, 4, E], BF16, tag="w")
                    nc.sync.dma_start(
                        out=w2_sb,
                        in_=d_w2[l][ch * 512:(ch + 1) * 512, :]
                        .rearrange("(m p) e -> p m e", p=128))
                    for m in range(4):
                        mt = ch * 4 + m
                        for e in range(ET):
                            nc.tensor.matmul(
                                ps_w2[e][:, 0:TOK],
                                w2_sb[:, m, e * 128:(e + 1) * 128],
                                mT[:, mt, :],
                                start=(mt == 0), stop=(mt == MT - 1))
                for e in range(ET):
                    t1 = tmpp.tile([128, TOK], F32, tag="t1")
                    nc.vector.tensor_scalar(t1, ps_w2[e][:, 0:TOK],
                                            b2_sb[:, e:e + 1], None, ALU.add)
                    nc.vector.tensor_add(xT[:, e, :], xT[:, e, :], t1)

            # ---- final LN -> xf^T (bf16), all-gather, LM head ----
            lnf_sb = smalls.tile([128, 2, ET], F32, tag="ln1")
            nc.sync.dma_start(out=lnf_sb, in_=d_lnf.rearrange("a p t -> p a t"))
            xbf = hp.tile([128, ET, TOK], BF16, tag="xb")
            for t in range(ET):
                nc.scalar.copy(xbf[:, t, :], xT[:, t, :])
            a_vec, b_vec = _ln_stats(nc, smalls, psum, ones,
                                     lambda t: xbf[:, t, :], ET, tmpp, E, eps_t)
            ab = _bcast(nc, bcp, a_vec)
            bb = _bcast(nc, bcp, b_vec)
            xfT = hp.tile([128, ET, TOK], BF16, tag="hT")
            for t in range(ET):
                t1 = tmpp.tile([128, TOK], F32, tag="t1")
                nc.vector.tensor_mul(t1, xT[:, t, :], ab)
                t2 = tmpp.tile([128, TOK], F32, tag="t2")
                nc.vector.tensor_sub(t2, t1, bb)
                nc.vector.tensor_scalar(xfT[:, t, :], t2,
                                        lnf_sb[:, 0, t:t + 1],
                                        lnf_sb[:, 1, t:t + 1],
                                        ALU.mult, ALU.add)

            xf_in = dram.tile([E, TOK], BF16)
            xf_out = dram.tile([GSZ * E, TOK], BF16)
            nc.gpsimd.dma_start(
                out=xf_in.rearrange("(e p) q -> p e q", p=128), in_=xfT)
            nc.gpsimd.collective_compute(
                "AllGather", ALU.bypass,
                replica_groups=[[0, 1, 2, 3], [4, 5, 6, 7]],
                ins=[xf_in[:].opt()], outs=[xf_out[:].opt()])
            # gathered xf^T: [E, T] with token cols in rank order
            xg = big.tile([128, ET, T], BF16, tag="uT")
            for r in range(GSZ):
                for t in range(ET):
                    nc.gpsimd.dma_start(
                        out=xg[:, t, r * TOK:(r + 1) * TOK],
                        in_=xf_out[r * E + t * 128: r * E + (t + 1) * 128, :])

            # gathered col block c holds global token block:
            #   rank = c//2; block = rank if c%2==0 else 7-rank
            for vc in range(NVC):
                wlm_sb = wts.tile([128, ET, VCHUNK], BF16, tag="w")
                nc.sync.dma_start(
                    out=wlm_sb,
                    in_=d_wlm[:, vc * VCHUNK:(vc + 1) * VCHUNK]
                    .rearrange("(e p) m -> p e m", p=128))
                blm_c = smalls.tile([1, VCHUNK], F32, tag="blmc")
                nc.sync.dma_start(
                    out=blm_c, in_=d_blm[:, vc * VCHUNK:(vc + 1) * VCHUNK])
                blm_b = bcp.tile([128, VCHUNK], F32, tag="blmb")
                nc.gpsimd.partition_broadcast(blm_b, blm_c)
                for c in range(NB):
                    rank, idx = c // 2, c % 2
                    blk = rank if idx == 0 else 7 - rank
                    ps = psum.tile([128, 512], F32, tag="ps")
                    for e in range(ET):
                        nc.tensor.matmul(
                            ps[:, 0:VCHUNK],
                            xg[:, e, c * 128:(c + 1) * 128],
                            wlm_sb[:, e, :],
                            start=(e == 0), stop=(e == ET - 1))
                    ot = outp.tile([128, VCHUNK], F32, tag="o")
                    nc.vector.tensor_add(ot, ps[:, 0:VCHUNK], blm_b)
                    nc.sync.dma_start(
                        out=d_out[blk * 128:(blk + 1) * 128,
                                  vc * VCHUNK:(vc + 1) * VCHUNK],
                        in_=ot)

    nc.compile()
    return nc


_CACHE = {}


def _get_program():
    if "nc" not in _CACHE:
        _CACHE["nc"] = build_program()
    return _CACHE["nc"]


def _prep_inputs(tokens, vocab_emb, pos_emb, ln1_g, ln1_b, Wq, Wk, Wv, Wo, bo,
                 ln2_g, ln2_b, W1, b1, lnm_g, lnm_b, W2, b2, lnf_g, lnf_b,
                 Wlm, blm):
    bf = ml_dtypes.bfloat16
    tokens = np.asarray(tokens)
    x0 = np.asarray(vocab_emb)[tokens] + np.asarray(pos_emb)[None, :T]

    def lnr(g, b, nt):  # [L?, dim] -> [L?, 2, 128, nt]
        g = np.asarray(g, np.float32)
        b = np.asarray(b, np.float32)
        st = np.stack([g, b], axis=-2)  # [..., 2, dim]
        return np.ascontiguousarray(
            st.reshape(*st.shape[:-1], nt, 128).swapaxes(-1, -2))

    def br(b, nt):  # [L, dim] -> [L, 128, nt]
        b = np.asarray(b, np.float32)
        return np.ascontiguousarray(b.reshape(*b.shape[:-1], nt, 128)
                                    .swapaxes(-1, -2))

    wq = np.ascontiguousarray(np.asarray(Wq)[:, 0]).astype(bf)
    wk = np.ascontiguousarray(np.asarray(Wk)[:, 0]).astype(bf)
    wv = np.ascontiguousarray(np.asarray(Wv)[:, 0]).astype(bf)
    wo = np.asarray(Wo).astype(bf)
    w1 = np.asarray(W1).astype(bf)
    w2 = np.asarray(W2).astype(bf)
    wlm_f = np.asarray(Wlm)
    blm_f = np.asarray(blm, dtype=np.float32)
    ln1 = lnr(ln1_g, ln1_b, ET)
    ln2 = lnr(ln2_g, ln2_b, ET)
    lnm = lnr(lnm_g, lnm_b, MT)
    lnf = lnr(lnf_g, lnf_b, ET)[0] if False else lnr(
        np.asarray(lnf_g)[None], np.asarray(lnf_b)[None], ET)[0]
    bo_r = br(bo, ET)
    b1_r = br(b1, MT)
    b2_r = br(b2, ET)

    in_maps = []
    for c in range(NCORES):
        g, r = c // GSZ, c % GSZ
        blocks = [r, 7 - r]
        rows = np.concatenate([np.arange(b * 128, (b + 1) * 128)
                               for b in blocks])
        x0T = np.ascontiguousarray(x0[g][rows].T.astype(np.float32))
        # mask [T(gathered key order), 256(own q)]
        key_pos = np.concatenate(
            [np.arange(rr * 128, rr * 128 + 128) if idx == 0 else
             np.arange((7 - rr) * 128, (7 - rr) * 128 + 128)
             for rr in range(GSZ) for idx in range(2)])
        q_pos = rows
        mask = (key_pos[:, None] <= q_pos[None, :]).astype(bf)
        in_maps.append({
            "x0T": x0T,
            "mask": mask,
            "wq": wq, "wk": wk, "wv": wv, "wo": wo, "w1": w1, "w2": w2,
            "ln1": ln1, "ln2": ln2, "lnm": lnm, "lnf": lnf,
            "bo": bo_r, "b1": b1_r, "b2": b2_r,
            "wlm": np.ascontiguousarray(
                wlm_f[:, r * VS:(r + 1) * VS]).astype(bf),
            "blm": np.ascontiguousarray(blm_f[r * VS:(r + 1) * VS])[None, :],
        })
    return in_maps


def kernel(**inputs):
    nc = _get_program()
    in_maps = _prep_inputs(**inputs)
    res = run_bass_kernel_spmd(nc, in_maps, core_ids=list(range(NCORES)))
    out = np.empty((B, T, V), np.float32)
    for c in range(NCORES):
        g, r = c // GSZ, c % GSZ
        out[g, :, r * VS:(r + 1) * VS] = res.results[c]["logits"]
    return out


if __name__ == "__main__":
    import reference
    inputs = {k: np.asarray(v) for k, v in reference.setup_inputs().items()}
    got = kernel(**inputs)
    want = np.asarray(reference.reference(**reference.setup_inputs()))
    denom = np.abs(want).max()
    err = np.abs(got - want).max() / denom
    print("max abs err:", np.abs(got - want).max(), "rel:", err)


# revision 12
# speedup vs baseline: 190.3511x; 190.3511x over previous
"""GPT-2 (V=32000, E=1024, H=1, HS=1024, L=4, T=1024, B=2) forward on 8 trn2 NeuronCores.

Sharding: 2 data-parallel groups of 4 cores (one per batch element). Within a
group, sequence is sharded "zigzag": rank r owns token blocks {r, 7-r} (128
tokens each).  Per layer each core computes q/k/v for its own 256 tokens,
all-gathers K^T and V inside its group, and computes attention rows + MLP for
its own tokens.  The LM head is vocab-sharded within the group (rank r owns
vocab columns [r*8000, (r+1)*8000)) after an all-gather of the final
activations.

All activations live in TRANSPOSED layout [feature-partition, token-free] so
every matmul is transpose-free; LayerNorm stats over the feature (partition)
dim are computed with ones-vector matmuls; per-token stats are applied via
partition_broadcast tiles.  Weights are cast to bf16 on host; accumulation in
fp32 PSUM.
"""
import sys

sys.path.insert(0, "/opt/trn_rl_repo")

import numpy as np
import ml_dtypes

import concourse.bass as bass
import concourse.tile as tile
from concourse import bacc, mybir
from concourse.bass_utils import run_bass_kernel_spmd

BF16 = mybir.dt.bfloat16
F32 = mybir.dt.float32
AF = mybir.ActivationFunctionType
ALU = mybir.AluOpType

V, E, H, HS, L, T, B = 32000, 1024, 1, 1024, 4, 1024, 2
EPS = 1e-5
SCALE = (H * HS) ** -0.5
NCORES = 8
GSZ = 4          # cores per group
NB = T // 128    # 8 token blocks per batch
VS = V // GSZ    # vocab shard per core = 8000
VCHUNK = 500     # vocab columns per psum tile (16 chunks)
NVC = VS // VCHUNK
ET = E // 128    # 8 feature tiles
MT = 4 * E // 128  # 32 mlp tiles
TOK = 256        # own tokens per core


def _ln_stats(nc, pool, psum, ones, src_bf, ntiles, sq_tile_pool, dim,
              eps_t=None):
    """Cross-partition LN stats over `ntiles` [128, TOK] bf16 tiles.

    Returns (a_vec, b_vec): [1, TOK] fp32 sbuf tiles with
    h = x * a - b  (before the per-feature affine)."""
    ps_sum = psum.tile([128, 512], F32, tag="ps")
    ps_sq = psum.tile([128, 512], F32, tag="ps")
    for t in range(ntiles):
        sq = sq_tile_pool.tile([128, TOK], BF16, tag="sq")
        nc.scalar.square(sq, src_bf(t))
        nc.tensor.matmul(ps_sum[0:1, 0:TOK], ones, src_bf(t),
                         start=(t == 0), stop=(t == ntiles - 1))
        nc.tensor.matmul(ps_sq[0:1, 0:TOK], ones, sq,
                         start=(t == 0), stop=(t == ntiles - 1))
    mean = pool.tile([1, TOK], F32, tag="stat")
    msq = pool.tile([1, TOK], F32, tag="stat")
    nc.scalar.mul(mean, ps_sum[0:1, 0:TOK], 1.0 / dim)
    nc.scalar.mul(msq, ps_sq[0:1, 0:TOK], 1.0 / dim)
    var = pool.tile([1, TOK], F32, tag="stat")
    nc.vector.tensor_mul(var, mean, mean)
    nc.vector.tensor_sub(var, msq, var)
    sd = pool.tile([1, TOK], F32, tag="stat")
    nc.scalar.activation(sd, var, AF.Sqrt, bias=eps_t)
    a_vec = pool.tile([1, TOK], F32, tag="stat")
    nc.vector.reciprocal(a_vec, sd)
    b_vec = pool.tile([1, TOK], F32, tag="stat")
    nc.vector.tensor_mul(b_vec, mean, a_vec)
    return a_vec, b_vec


def _bcast(nc, pool, vec, tag="bc"):
    out = pool.tile([128, TOK], F32, tag=tag)
    nc.gpsimd.partition_broadcast(out, vec)
    return out


def build_program(sim=False):
    nc = bacc.Bacc("TRN2", target_bir_lowering=False, debug=False,
                   num_devices=1 if sim else NCORES)

    def allgather(cin, cout, nslots):
        if sim:
            # timing-model stand-in: local copies into each slot
            rows = cin.shape[0]
            for r in range(nslots):
                nc.gpsimd.dma_start(out=cout[r * rows:(r + 1) * rows], in_=cin)
        else:
            nc.gpsimd.collective_compute(
                "AllGather", ALU.bypass,
                replica_groups=[[0, 1, 2, 3], [4, 5, 6, 7]],
                ins=[cin[:].opt()], outs=[cout[:].opt()])

    d_x0 = nc.dram_tensor("x0T", [E, TOK], F32, kind="ExternalInput").ap()
    d_mask = nc.dram_tensor("mask", [T, TOK], BF16, kind="ExternalInput").ap()
    d_wq = nc.dram_tensor("wq", [L, E, HS], BF16, kind="ExternalInput").ap()
    d_wk = nc.dram_tensor("wk", [L, E, HS], BF16, kind="ExternalInput").ap()
    d_wv = nc.dram_tensor("wv", [L, E, HS], BF16, kind="ExternalInput").ap()
    d_wo = nc.dram_tensor("wo", [L, HS, E], BF16, kind="ExternalInput").ap()
    d_w1 = nc.dram_tensor("w1", [L, E, 4 * E], BF16, kind="ExternalInput").ap()
    d_w2 = nc.dram_tensor("w2", [L, 4 * E, E], BF16, kind="ExternalInput").ap()
    # per-partition-tile reshaped affines/biases: [*, 128, ntiles]
    d_ln1 = nc.dram_tensor("ln1", [L, 2, 128, ET], F32, kind="ExternalInput").ap()
    d_ln2 = nc.dram_tensor("ln2", [L, 2, 128, ET], F32, kind="ExternalInput").ap()
    d_lnm = nc.dram_tensor("lnm", [L, 2, 128, MT], F32, kind="ExternalInput").ap()
    d_lnf = nc.dram_tensor("lnf", [2, 128, ET], F32, kind="ExternalInput").ap()
    d_bo = nc.dram_tensor("bo", [L, 128, ET], F32, kind="ExternalInput").ap()
    d_b1 = nc.dram_tensor("b1", [L, 128, MT], F32, kind="ExternalInput").ap()
    d_b2 = nc.dram_tensor("b2", [L, 128, ET], F32, kind="ExternalInput").ap()
    d_wlm = nc.dram_tensor("wlm", [E, VS], BF16, kind="ExternalInput").ap()
    d_blm = nc.dram_tensor("blm", [1, VS], F32, kind="ExternalInput").ap()
    d_out = nc.dram_tensor("logits", [T, VS], F32, kind="ExternalOutput").ap()

    with tile.TileContext(nc) as tc:
        import contextlib
        ctx = contextlib.ExitStack()
        with ctx:
            const = ctx.enter_context(tc.tile_pool(name="const", bufs=1))
            smalls = ctx.enter_context(tc.tile_pool(name="smalls", bufs=6))
            bcp = ctx.enter_context(tc.tile_pool(name="bcast", bufs=4))
            hp = ctx.enter_context(tc.tile_pool(name="acts", bufs=1))
            kqv = ctx.enter_context(tc.tile_pool(name="kqv", bufs=1))
            big = ctx.enter_context(tc.tile_pool(name="big", bufs=1))
            wts = ctx.enter_context(tc.tile_pool(name="wts", bufs=3))
            tmpp = ctx.enter_context(tc.tile_pool(name="tmpp", bufs=6))
            outp = ctx.enter_context(tc.tile_pool(name="outp", bufs=3))
            psum = ctx.enter_context(tc.tile_pool(name="psum", bufs=8,
                                                  space="PSUM"))
            dram = ctx.enter_context(tc.tile_pool(name="dram", bufs=1,
                                                  space="DRAM"))

            ones = const.tile([128, 1], BF16)
            nc.vector.memset(ones, 1.0)
            eps_t = const.tile([1, 1], F32)
            nc.vector.memset(eps_t, EPS)

            mask_sb = const.tile([128, NB, TOK], BF16)
            nc.sync.dma_start(out=mask_sb,
                              in_=d_mask.rearrange("(n p) q -> p n q", p=128))

            # residual stream x^T, fp32, [E, 256]
            xT = const.tile([128, ET, TOK], F32)
            nc.sync.dma_start(out=xT,
                              in_=d_x0.rearrange("(e p) q -> p e q", p=128))

            # gathered K^T [HS, T] and V [T, HS], bf16
            kt_g = big.tile([128, ET, T], BF16)
            v_g = big.tile([128, NB, HS], BF16)

            for l in range(L):
                ln1_sb = smalls.tile([128, 2, ET], F32, tag="ln1")
                nc.sync.dma_start(out=ln1_sb, in_=d_ln1[l].rearrange("a p t -> p a t"))
                ln2_sb = smalls.tile([128, 2, ET], F32, tag="ln2")
                nc.sync.dma_start(out=ln2_sb, in_=d_ln2[l].rearrange("a p t -> p a t"))
                lnm_sb = smalls.tile([128, 2, MT], F32, tag="lnm")
                nc.sync.dma_start(out=lnm_sb, in_=d_lnm[l].rearrange("a p t -> p a t"))
                bo_sb = smalls.tile([128, ET], F32, tag="bo")
                nc.sync.dma_start(out=bo_sb, in_=d_bo[l])
                b1_sb = smalls.tile([128, MT], F32, tag="b1")
                nc.sync.dma_start(out=b1_sb, in_=d_b1[l])
                b2_sb = smalls.tile([128, ET], F32, tag="b2")
                nc.sync.dma_start(out=b2_sb, in_=d_b2[l])

                # ---- LN1 -> h^T (bf16) ----
                xb = hp.tile([128, ET, TOK], BF16, tag="xb")
                for t in range(ET):
                    nc.scalar.copy(xb[:, t, :], xT[:, t, :])
                a_vec, b_vec = _ln_stats(nc, smalls, psum, ones,
                                         lambda t: xb[:, t, :], ET, tmpp, E, eps_t)
                ab = _bcast(nc, bcp, a_vec)
                bb = _bcast(nc, bcp, b_vec)
                hT = hp.tile([128, ET, TOK], BF16, tag="hT")
                for t in range(ET):
                    t1 = tmpp.tile([128, TOK], F32, tag="t1")
                    nc.vector.tensor_mul(t1, xT[:, t, :], ab)
                    t2 = tmpp.tile([128, TOK], F32, tag="t2")
                    nc.vector.tensor_sub(t2, t1, bb)
                    nc.vector.tensor_scalar(hT[:, t, :], t2,
                                            ln1_sb[:, 0, t:t + 1],
                                            ln1_sb[:, 1, t:t + 1],
                                            ALU.mult, ALU.add)

                # ---- k^T = (h Wk)^T, q^T likewise; v in row layout ----
                kT = kqv.tile([128, ET, TOK], BF16, tag="kT")
                qT = kqv.tile([128, ET, TOK], BF16, tag="qT")
                for half in range(2):
                    wk_sb = wts.tile([128, ET, HS // 2], BF16, tag="w")
                    nc.sync.dma_start(
                        out=wk_sb,
                        in_=d_wk[l][:, half * 512:(half + 1) * 512]
                        .rearrange("(e p) m -> p e m", p=128))
                    for m in range(4):
                        ps = psum.tile([128, 512], F32, tag="ps")
                        for e in range(ET):
                            nc.tensor.matmul(
                                ps[:, 0:TOK],
                                wk_sb[:, e, m * 128:(m + 1) * 128],
                                hT[:, e, :],
                                start=(e == 0), stop=(e == ET - 1))
                        nc.vector.tensor_copy(out=kT[:, half * 4 + m, :],
                                              in_=ps[:, 0:TOK])
                for half in range(2):
                    wq_sb = wts.tile([128, ET, HS // 2], BF16, tag="w")
                    nc.sync.dma_start(
                        out=wq_sb,
                        in_=d_wq[l][:, half * 512:(half + 1) * 512]
                        .rearrange("(e p) m -> p e m", p=128))
                    for m in range(4):
                        ps = psum.tile([128, 512], F32, tag="ps")
                        for e in range(ET):
                            nc.tensor.matmul(
                                ps[:, 0:TOK],
                                wq_sb[:, e, m * 128:(m + 1) * 128],
                                hT[:, e, :],
                                start=(e == 0), stop=(e == ET - 1))
                        nc.vector.tensor_copy(out=qT[:, half * 4 + m, :],
                                              in_=ps[:, 0:TOK])
                v_sb = kqv.tile([128, 2, HS], BF16, tag="v")
                for half in range(2):
                    wv_sb = wts.tile([128, ET, HS // 2], BF16, tag="w")
                    nc.sync.dma_start(
                        out=wv_sb,
                        in_=d_wv[l][:, half * 512:(half + 1) * 512]
                        .rearrange("(e p) m -> p e m", p=128))
                    for blk in range(2):
                        ps = psum.tile([128, 512], F32, tag="ps")
                        for e in range(ET):
                            nc.tensor.matmul(
                                ps,
                                hT[:, e, blk * 128:(blk + 1) * 128],
                                wv_sb[:, e, :],
                                start=(e == 0), stop=(e == ET - 1))
                        nc.vector.tensor_copy(
                            out=v_sb[:, blk, half * 512:(half + 1) * 512],
                            in_=ps)

                # ---- all-gather K^T and V within group of 4 ----
                kb_in = dram.tile([E, TOK], BF16)
                kb_out = dram.tile([GSZ * E, TOK], BF16)
                nc.gpsimd.dma_start(
                    out=kb_in.rearrange("(e p) q -> p e q", p=128), in_=kT)
                allgather(kb_in, kb_out, GSZ)
                vb_in = dram.tile([TOK, HS], BF16)
                vb_out = dram.tile([GSZ * TOK, HS], BF16)
                nc.gpsimd.dma_start(
                    out=vb_in.rearrange("(b p) h -> p b h", p=128), in_=v_sb)
                allgather(vb_in, vb_out, GSZ)
                # read back gathered K^T: rank r tile t -> cols r*256..
                for r in range(GSZ):
                    for t in range(ET):
                        nc.gpsimd.dma_start(
                            out=kt_g[:, t, r * TOK:(r + 1) * TOK],
                            in_=kb_out[r * E + t * 128: r * E + (t + 1) * 128, :])
                for kb in range(NB):
                    nc.gpsimd.dma_start(
                        out=v_g[:, kb, :],
                        in_=vb_out[kb * 128:(kb + 1) * 128, :])

                # ---- scores^T, masked exp, normalizer, o^T ----
                aT = hp.tile([128, NB, TOK], BF16, tag="aT")
                ps_r = psum.tile([128, 512], F32, tag="ps")
                for kb in range(NB):
                    ps = psum.tile([128, 512], F32, tag="ps")
                    for t in range(ET):
                        nc.tensor.matmul(
                            ps[:, 0:TOK],
                            kt_g[:, t, kb * 128:(kb + 1) * 128],
                            qT[:, t, :],
                            start=(t == 0), stop=(t == ET - 1))
                    nc.scalar.activation(aT[:, kb, :], ps[:, 0:TOK],
                                         AF.Exp, scale=SCALE)
                    nc.vector.tensor_mul(aT[:, kb, :], aT[:, kb, :],
                                         mask_sb[:, kb, :])
                    nc.tensor.matmul(ps_r[0:1, 0:TOK], ones, aT[:, kb, :],
                                     start=(kb == 0), stop=(kb == NB - 1))
                rinv = smalls.tile([1, TOK], F32, tag="stat")
                nc.vector.reciprocal(rinv, ps_r[0:1, 0:TOK])
                rb = _bcast(nc, bcp, rinv)
                oT = kqv.tile([128, ET, TOK], BF16, tag="oT")
                for m in range(ET):
                    ps = psum.tile([128, 512], F32, tag="ps")
                    for kb in range(NB):
                        nc.tensor.matmul(
                            ps[:, 0:TOK],
                            v_g[:, kb, m * 128:(m + 1) * 128],
                            aT[:, kb, :],
                            start=(kb == 0), stop=(kb == NB - 1))
                    nc.vector.tensor_mul(oT[:, m, :], ps[:, 0:TOK], rb)

                # ---- x += o Wo + bo ----
                for half in range(2):
                    wo_sb = wts.tile([128, ET, E // 2], BF16, tag="w")
                    nc.sync.dma_start(
                        out=wo_sb,
                        in_=d_wo[l][:, half * 512:(half + 1) * 512]
                        .rearrange("(h p) m -> p h m", p=128))
                    for m in range(4):
                        et = half * 4 + m
                        ps = psum.tile([128, 512], F32, tag="ps")
                        for h in range(ET):
                            nc.tensor.matmul(
                                ps[:, 0:TOK],
                                wo_sb[:, h, m * 128:(m + 1) * 128],
                                oT[:, h, :],
                                start=(h == 0), stop=(h == ET - 1))
                        t1 = tmpp.tile([128, TOK], F32, tag="t1")
                        nc.vector.tensor_scalar(t1, ps[:, 0:TOK],
                                                bo_sb[:, et:et + 1], None,
                                                ALU.add)
                        nc.vector.tensor_add(xT[:, et, :], xT[:, et, :], t1)

                # ---- LN2 -> h2^T ----
                xb2 = hp.tile([128, ET, TOK], BF16, tag="xb")
                for t in range(ET):
                    nc.scalar.copy(xb2[:, t, :], xT[:, t, :])
                a_vec, b_vec = _ln_stats(nc, smalls, psum, ones,
                                         lambda t: xb2[:, t, :], ET, tmpp, E, eps_t)
                ab = _bcast(nc, bcp, a_vec)
                bb = _bcast(nc, bcp, b_vec)
                h2T = hp.tile([128, ET, TOK], BF16, tag="hT")
                for t in range(ET):
                    t1 = tmpp.tile([128, TOK], F32, tag="t1")
                    nc.vector.tensor_mul(t1, xT[:, t, :], ab)
                    t2 = tmpp.tile([128, TOK], F32, tag="t2")
                    nc.vector.tensor_sub(t2, t1, bb)
                    nc.vector.tensor_scalar(h2T[:, t, :], t2,
                                            ln2_sb[:, 0, t:t + 1],
                                            ln2_sb[:, 1, t:t + 1],
                                            ALU.mult, ALU.add)

                # ---- u^T = (h2 W1 + b1)^T ----
                uT = big.tile([128, MT, TOK], BF16, tag="uT")
                for ch in range(8):   # 8 chunks of 512 mlp cols (4 m-tiles)
                    w1_sb = wts.tile([128, ET, 512], BF16, tag="w")
                    nc.sync.dma_start(
                        out=w1_sb,
                        in_=d_w1[l][:, ch * 512:(ch + 1) * 512]
                        .rearrange("(e p) m -> p e m", p=128))
                    for m in range(4):
                        mt = ch * 4 + m
                        ps = psum.tile([128, 512], F32, tag="ps")
                        for e in range(ET):
                            nc.tensor.matmul(
                                ps[:, 0:TOK],
                                w1_sb[:, e, m * 128:(m + 1) * 128],
                                hT if False else h2T[:, e, :],
                                start=(e == 0), stop=(e == ET - 1))
                        nc.vector.tensor_scalar(uT[:, mt, :], ps[:, 0:TOK],
                                                b1_sb[:, mt:mt + 1], None,
                                                ALU.add)

                # ---- m^T = gelu(lnm(u)) ----
                a_vec, b_vec = _ln_stats(nc, smalls, psum, ones,
                                         lambda t: uT[:, t, :], MT, tmpp, 4 * E, eps_t)
                ab = _bcast(nc, bcp, a_vec)
                bb = _bcast(nc, bcp, b_vec)
                mT = big.tile([128, MT, TOK], BF16, tag="mT")
                for t in range(MT):
                    t1 = tmpp.tile([128, TOK], F32, tag="t1")
                    nc.vector.tensor_mul(t1, uT[:, t, :], ab)
                    t2 = tmpp.tile([128, TOK], F32, tag="t2")
                    nc.vector.tensor_sub(t2, t1, bb)
                    nc.scalar.activation(mT[:, t, :], t2, AF.Gelu,
                                         bias=lnm_sb[:, 1, t:t + 1],
                                         scale=lnm_sb[:, 0, t:t + 1])

                # ---- x += m W2 + b2 (m-major accumulation) ----
                ps_w2 = [psum.tile([128, 512], F32, tag="ps", name=f"psw2_{e}")
                         for e in range(ET)]
                for ch in range(8):   # 8 chunks of 512 contraction rows
                    w2_sb = wts.tile([128, 4, E], BF16, tag="w")
                    nc.sync.dma_start(
                        out=w2_sb,
                        in_=d_w2[l][ch * 512:(ch + 1) * 512, :]
                        .rearrange("(m p) e -> p m e", p=128))
                    for m in range(4):
                        mt = ch * 4 + m
                        for e in range(ET):
                            nc.tensor.matmul(
                                ps_w2[e][:, 0:TOK],
                                w2_sb[:, m, e * 128:(e + 1) * 128],
                                mT[:, mt, :],
                                start=(mt == 0), stop=(mt == MT - 1))
                for e in range(ET):
                    t1 = tmpp.tile([128, TOK], F32, tag="t1")
                    nc.vector.tensor_scalar(t1, ps_w2[e][:, 0:TOK],
                                            b2_sb[:, e:e + 1], None, ALU.add)
                    nc.vector.tensor_add(xT[:, e, :], xT[:, e, :], t1)

            # ---- final LN -> xf^T (bf16), all-gather, LM head ----
            lnf_sb = smalls.tile([128, 2, ET], F32, tag="ln1")
            nc.sync.dma_start(out=lnf_sb, in_=d_lnf.rearrange("a p t -> p a t"))
            xbf = hp.tile([128, ET, TOK], BF16, tag="xb")
            for t in range(ET):
                nc.scalar.copy(xbf[:, t, :], xT[:, t, :])
            a_vec, b_vec = _ln_stats(nc, smalls, psum, ones,
                                     lambda t: xbf[:, t, :], ET, tmpp, E, eps_t)
            ab = _bcast(nc, bcp, a_vec)
            bb = _bcast(nc, bcp, b_vec)
            xfT = hp.tile([128, ET, TOK], BF16, tag="hT")
            for t in range(ET):
                t1 = tmpp.tile([128, TOK], F32, tag="t1")
                nc.vector.tensor_mul(t1, xT[:, t, :], ab)
                t2 = tmpp.tile([128, TOK], F32, tag="t2")
                nc.vector.tensor_sub(t2, t1, bb)
                nc.vector.tensor_scalar(xfT[:, t, :], t2,
                                        lnf_sb[:, 0, t:t + 1],
                                        lnf_sb[:, 1, t:t + 1],
                                        ALU.mult, ALU.add)

            xf_in = dram.tile([E, TOK], BF16)
            xf_out = dram.tile([GSZ * E, TOK], BF16)
            nc.gpsimd.dma_start(
                out=xf_in.rearrange("(e p) q -> p e q", p=128), in_=xfT)
            allgather(xf_in, xf_out, GSZ)
            # gathered xf^T: [E, T] with token cols in rank order
            xg = big.tile([128, ET, T], BF16, tag="uT")
            for r in range(GSZ):
                for t in range(ET):
                    nc.gpsimd.dma_start(
                        out=xg[:, t, r * TOK:(r + 1) * TOK],
                        in_=xf_out[r * E + t * 128: r * E + (t + 1) * 128, :])

            # gathered col block c holds global token block:
            #   rank = c//2; block = rank if c%2==0 else 7-rank
            for vc in range(NVC):
                wlm_sb = wts.tile([128, ET, VCHUNK], BF16, tag="w")
                nc.sync.dma_start(
                    out=wlm_sb,
                    in_=d_wlm[:, vc * VCHUNK:(vc + 1) * VCHUNK]
                    .rearrange("(e p) m -> p e m", p=128))
                blm_c = smalls.tile([1, VCHUNK], F32, tag="blmc")
                nc.sync.dma_start(
                    out=blm_c, in_=d_blm[:, vc * VCHUNK:(vc + 1) * VCHUNK])
                blm_b = bcp.tile([128, VCHUNK], F32, tag="blmb")
                nc.gpsimd.partition_broadcast(blm_b, blm_c)
                for c in range(NB):
                    rank, idx = c // 2, c % 2
                    blk = rank if idx == 0 else 7 - rank
                    ps = psum.tile([128, 512], F32, tag="ps")
                    for e in range(ET):
                        nc.tensor.matmul(
                            ps[:, 0:VCHUNK],
                            xg[:, e, c * 128:(c + 1) * 128],
                            wlm_sb[:, e, :],
                            start=(e == 0), stop=(e == ET - 1))
                    ot = outp.tile([128, VCHUNK], F32, tag="o")
                    nc.vector.tensor_add(ot, ps[:, 0:VCHUNK], blm_b)
                    nc.sync.dma_start(
                        out=d_out[blk * 128:(blk + 1) * 128,
                                  vc * VCHUNK:(vc + 1) * VCHUNK],
                        in_=ot)

    nc.compile()
    return nc


_CACHE = {}


def _get_program():
    if "nc" not in _CACHE:
        _CACHE["nc"] = build_program()
    return _CACHE["nc"]


def _prep_inputs(tokens, vocab_emb, pos_emb, ln1_g, ln1_b, Wq, Wk, Wv, Wo, bo,
                 ln2_g, ln2_b, W1, b1, lnm_g, lnm_b, W2, b2, lnf_g, lnf_b,
                 Wlm, blm):
    bf = ml_dtypes.bfloat16
    tokens = np.asarray(tokens)
    x0 = np.asarray(vocab_emb)[tokens] + np.asarray(pos_emb)[None, :T]

    def lnr(g, b, nt):  # [L?, dim] -> [L?, 2, 128, nt]
        g = np.asarray(g, np.float32)
        b = np.asarray(b, np.float32)
        st = np.stack([g, b], axis=-2)  # [..., 2, dim]
        return np.ascontiguousarray(
            st.reshape(*st.shape[:-1], nt, 128).swapaxes(-1, -2))

    def br(b, nt):  # [L, dim] -> [L, 128, nt]
        b = np.asarray(b, np.float32)
        return np.ascontiguousarray(b.reshape(*b.shape[:-1], nt, 128)
                                    .swapaxes(-1, -2))

    wq = np.ascontiguousarray(np.asarray(Wq)[:, 0]).astype(bf)
    wk = np.ascontiguousarray(np.asarray(Wk)[:, 0]).astype(bf)
    wv = np.ascontiguousarray(np.asarray(Wv)[:, 0]).astype(bf)
    wo = np.asarray(Wo).astype(bf)
    w1 = np.asarray(W1).astype(bf)
    w2 = np.asarray(W2).astype(bf)
    wlm_f = np.asarray(Wlm)
    blm_f = np.asarray(blm, dtype=np.float32)
    ln1 = lnr(ln1_g, ln1_b, ET)
    ln2 = lnr(ln2_g, ln2_b, ET)
    lnm = lnr(lnm_g, lnm_b, MT)
    lnf = lnr(lnf_g, lnf_b, ET)[0] if False else lnr(
        np.asarray(lnf_g)[None], np.asarray(lnf_b)[None], ET)[0]
    bo_r = br(bo, ET)
    b1_r = br(b1, MT)
    b2_r = br(b2, ET)

    in_maps = []
    for c in range(NCORES):
        g, r = c // GSZ, c % GSZ
        blocks = [r, 7 - r]
        rows = np.concatenate([np.arange(b * 128, (b + 1) * 128)
                               for b in blocks])
        x0T = np.ascontiguousarray(x0[g][rows].T.astype(np.float32))
        # mask [T(gathered key order), 256(own q)]
        key_pos = np.concatenate(
            [np.arange(rr * 128, rr * 128 + 128) if idx == 0 else
             np.arange((7 - rr) * 128, (7 - rr) * 128 + 128)
             for rr in range(GSZ) for idx in range(2)])
        q_pos = rows
        mask = (key_pos[:, None] <= q_pos[None, :]).astype(bf)
        in_maps.append({
            "x0T": x0T,
            "mask": mask,
            "wq": wq, "wk": wk, "wv": wv, "wo": wo, "w1": w1, "w2": w2,
            "ln1": ln1, "ln2": ln2, "lnm": lnm, "lnf": lnf,
            "bo": bo_r, "b1": b1_r, "b2": b2_r,
            "wlm": np.ascontiguousarray(
                wlm_f[:, r * VS:(r + 1) * VS]).astype(bf),
            "blm": np.ascontiguousarray(blm_f[r * VS:(r + 1) * VS])[None, :],
        })
    return in_maps


def kernel(**inputs):
    nc = _get_program()
    in_maps = _prep_inputs(**inputs)
    res = run_bass_kernel_spmd(nc, in_maps, core_ids=list(range(NCORES)))
    out = np.empty((B, T, V), np.float32)
    for c in range(NCORES):
        g, r = c // GSZ, c % GSZ
        out[g, :, r * VS:(r + 1) * VS] = res.results[c]["logits"]
    return out


if __name__ == "__main__":
    import reference
    inputs = {k: np.asarray(v) for k, v in reference.setup_inputs().items()}
    got = kernel(**inputs)
    want = np.asarray(reference.reference(**reference.setup_inputs()))
    denom = np.abs(want).max()
    err = np.abs(got - want).max() / denom
    print("max abs err:", np.abs(got - want).max(), "rel:", err)
